# revision 6
# baseline (speedup 1.0000x reference)
"""Distributed Trainium2 kernel for a single causal attention head.

Problem (hardcoded): B=4, S=2048, D_MODEL=1024, HEAD_DIM=64, fp32 inputs.
    q = query @ Wq + bq ; k = key @ Wk + bk ; v = value @ Wv + bv
    scores = q k^T / sqrt(H) ; masked softmax ; out = att @ v

Sharding (8 NeuronCores): core c = (b, h) with b = c//2, h = c%2.
Each core owns 4 query chunks of 256 rows of batch b and HALF the
key/value rows (h-th 1024).  k/v are projected locally on their owning
core and exchanged between the pair with an AllGather (halves input DMA
and projection FLOPs).  To balance causal work with one SPMD program,
chunks are assigned h=0 -> {0,3,4,7}, h=1 -> {1,2,5,6}; the program has
4 slots with fixed k-extents of 4/8/12/16 j-tiles (j-tile = 128 keys).
Per-core differences are pure data: gathered query rows and host-built
predicate masks (from the real `mask` input) that zero attention
weights after exp.  Predicate slots cover j-tiles [4s, 4s+4) of slot s.

Device layout trick: query/key/value shards are passed TRANSPOSED
([D, S] bf16) so every matmul contracts over the partition dim with
operands in natural layout (no on-device input transposes):
  qT[h,i]    = Wq^T Xq^T    (lhsT=Wq chunk, rhs=XqT chunk)
  kT/vT[h,j] = W^T X^T      (own half; pair-AllGather -> full kT; v is
                             read back with DMA-transpose to [j, h])
  sT[j,i]    = kT-tile as lhsT, rhs=qT        (scores transposed)
  att        = exp(sT * 0.125)  (ScalarE, PSUM->SBUF, bf16)
  oT[65,i]  += v_aug-tile as lhsT, rhs=att    (v_aug ones column ->
                                               row 64 = softmax denom)
Final: PE-transpose [65,128] blocks, scale by reciprocal of column 64,
single DMA out as [i, 64] fp32.
"""

import os

import numpy as np
import ml_dtypes

import concourse.bass as bass
import concourse.tile as tile
from concourse import bacc, mybir
from concourse.bass import ds
from concourse.bass_utils import run_bass_kernel_spmd
from concourse.masks import make_identity

B, S, D, H = 4, 2048, 1024, 64
P = 128
NCORES = 8
CHUNK = 256               # query rows per slot
NSLOTS = 8 // 2           # 4 slots per core (4 x 256 = 1024 q rows)
NQ = NSLOTS * CHUNK       # 1024
SHALF = S // 2            # k/v rows owned per core
JT = S // P               # 16 j-tiles of 128 keys
DCH = D // P              # 8 contraction chunks
FP = mybir.dt.float32
BF = mybir.dt.bfloat16
U8 = mybir.dt.uint8
BF_NP = ml_dtypes.bfloat16

# causal variant: slot extents (j-tiles) and per-h chunk assignment
CAUSAL_EXTENTS = (4, 8, 12, 16)
CAUSAL_CHUNKS = {0: (0, 3, 4, 7), 1: (1, 2, 5, 6)}
CAUSAL_MASKED = [(s, jt) for s in range(4) for jt in range(4 * s, 4 * s + 4)]

FULL_EXTENTS = (16, 16, 16, 16)
FULL_CHUNKS = {0: (0, 1, 2, 3), 1: (4, 5, 6, 7)}
FULL_MASKED = [(s, jt) for s in range(4) for jt in range(16)]

LAST_RESULTS = None
_PROGRAM_CACHE = {}


def _build_program(extents, masked_slots):
    """Build the SPMD Bass program (identical on all 8 cores)."""
    nc = bacc.Bacc("TRN2", target_bir_lowering=False, debug=False,
                   num_devices=NCORES)

    qT_d = nc.dram_tensor("qT", [D, NQ], BF, kind="ExternalInput").ap()
    kT_d = nc.dram_tensor("kT", [D, SHALF], BF, kind="ExternalInput").ap()
    vT_d = nc.dram_tensor("vT", [D, SHALF], BF, kind="ExternalInput").ap()
    wq_d = nc.dram_tensor("wq", [D, H], BF, kind="ExternalInput").ap()
    wk_d = nc.dram_tensor("wk", [D, H], BF, kind="ExternalInput").ap()
    wv_d = nc.dram_tensor("wv", [D, H], BF, kind="ExternalInput").ap()
    bq_d = nc.dram_tensor("bq", [H, 1], FP, kind="ExternalInput").ap()
    bk_d = nc.dram_tensor("bk", [H, 1], FP, kind="ExternalInput").ap()
    bv_d = nc.dram_tensor("bv", [H, 1], FP, kind="ExternalInput").ap()
    nmask = len(masked_slots)
    pred_d = nc.dram_tensor("pred", [nmask, P, CHUNK], U8,
                            kind="ExternalInput").ap()
    out_d = nc.dram_tensor("out", [NQ, H], FP, kind="ExternalOutput").ap()
    debug = bool(os.environ.get("BASS_DEBUG_DUMP"))
    if debug:
        dbg_k = nc.dram_tensor("dbg_k", [P, S], BF, kind="ExternalOutput").ap()
        dbg_v = nc.dram_tensor("dbg_v", [P, JT, H + 1], BF,
                               kind="ExternalOutput").ap()
        dbg_q = nc.dram_tensor("dbg_q", [P, NQ], BF,
                               kind="ExternalOutput").ap()

    groups = [[0, 1], [2, 3], [4, 5], [6, 7]]

    with tile.TileContext(nc) as tc:
        with (
            tc.tile_pool(name="const", bufs=1) as const,
            tc.tile_pool(name="resident", bufs=1) as res,
            tc.tile_pool(name="attp", bufs=4) as attp,
            tc.tile_pool(name="outp", bufs=2) as outp,
            tc.tile_pool(name="dram", bufs=1, space="DRAM") as dram,
            tc.tile_pool(name="pp", bufs=2, space="PSUM") as pp,
            tc.tile_pool(name="psc", bufs=3, space="PSUM") as psc,
            tc.tile_pool(name="pout", bufs=2, space="PSUM") as pout,
            tc.tile_pool(name="ptr", bufs=1, space="PSUM") as ptr,
        ):
            # ---- constants ----
            wq_sb = const.tile([P, DCH, H], BF, tag="wq")
            wk_sb = const.tile([P, DCH, H], BF, tag="wk")
            wv_sb = const.tile([P, DCH, H], BF, tag="wv")
            nc.sync.dma_start(wq_sb, wq_d.rearrange("(o p) h -> p o h", p=P))
            nc.sync.dma_start(wk_sb, wk_d.rearrange("(o p) h -> p o h", p=P))
            nc.sync.dma_start(wv_sb, wv_d.rearrange("(o p) h -> p o h", p=P))
            bq_sb = const.tile([H, 1], FP, tag="bq")
            bk_sb = const.tile([H, 1], FP, tag="bk")
            bv_sb = const.tile([H, 1], FP, tag="bv")
            nc.sync.dma_start(bq_sb, bq_d)
            nc.sync.dma_start(bk_sb, bk_d)
            nc.sync.dma_start(bv_sb, bv_d)
            zeros_sb = const.tile([P, CHUNK], BF, tag="zeros")
            nc.vector.memset(zeros_sb, 0.0)
            ident = const.tile([P, P], FP, tag="ident")
            make_identity(nc, ident)
            pred_sb = res.tile([P, nmask, CHUNK], U8, tag="pred")
            nc.sync.dma_start(pred_sb, pred_d.rearrange("t p f -> p t f"))

            # ---- resident inputs (k first: it gates the collective) ----
            xk_sb = res.tile([P, DCH, SHALF], BF, tag="xk")
            xv_sb = res.tile([P, DCH, SHALF], BF, tag="xv")
            xq_sb = res.tile([P, DCH, NQ], BF, tag="xq")
            kT_r = kT_d.rearrange("(o p) s -> p o s", p=P)
            vT_r = vT_d.rearrange("(o p) s -> p o s", p=P)
            qT_r = qT_d.rearrange("(o p) i -> p o i", p=P)
            for o in range(DCH):
                nc.sync.dma_start(xk_sb[:, o, :], kT_r[:, o, :])
            for o in range(DCH):
                nc.sync.dma_start(xv_sb[:, o, :], vT_r[:, o, :])
            for o in range(DCH):
                nc.sync.dma_start(xq_sb[:, o, :], qT_r[:, o, :])

            # ---- k/v half projections -> pair AllGather ----
            khalf_b = dram.tile([H, SHALF], BF)
            vhalf_b = dram.tile([H, SHALF], BF)
            kgath = dram.tile([2 * H, SHALF], BF)
            vgath = dram.tile([2 * H, SHALF], BF)
            khalf_sb = res.tile([H, SHALF], BF, tag="khalf")
            vhalf_sb = res.tile([H, SHALF], BF, tag="vhalf")
            for ic in range(SHALF // 512):
                pk = pp.tile([H, 512], FP, tag="pp")
                for d in range(DCH):
                    nc.tensor.matmul(pk, lhsT=wk_sb[:, d, :],
                                     rhs=xk_sb[:, d, ds(ic * 512, 512)],
                                     start=(d == 0), stop=(d == DCH - 1))
                nc.scalar.activation(khalf_sb[:, ds(ic * 512, 512)], pk,
                                     mybir.ActivationFunctionType.Identity,
                                     bias=bk_sb)
            nc.sync.dma_start(khalf_b[:], khalf_sb[:])
            nc.gpsimd.collective_compute(
                "AllGather", mybir.AluOpType.bypass, replica_groups=groups,
                ins=[khalf_b.opt()], outs=[kgath.opt()])
            for ic in range(SHALF // 512):
                pv = pp.tile([H, 512], FP, tag="pp")
                for d in range(DCH):
                    nc.tensor.matmul(pv, lhsT=wv_sb[:, d, :],
                                     rhs=xv_sb[:, d, ds(ic * 512, 512)],
                                     start=(d == 0), stop=(d == DCH - 1))
                nc.scalar.activation(vhalf_sb[:, ds(ic * 512, 512)], pv,
                                     mybir.ActivationFunctionType.Identity,
                                     bias=bv_sb)
            nc.sync.dma_start(vhalf_b[:], vhalf_sb[:])
            nc.gpsimd.collective_compute(
                "AllGather", mybir.AluOpType.bypass, replica_groups=groups,
                ins=[vhalf_b.opt()], outs=[vgath.opt()])

            # gathered k -> SBUF [h(pad 128), j]; v -> [j, h] via
            # DMA-transpose, plus ones column for the softmax denominator
            k_sb = res.tile([P, S], BF, tag="k")
            nc.vector.memset(k_sb[H:, :], 0.0)
            nc.sync.dma_start(k_sb[:H, 0:SHALF], kgath[0:H, :])
            nc.sync.dma_start(k_sb[:H, SHALF:S], kgath[H:2 * H, :])
            v_sb = res.tile([P, JT, H + 1], BF, tag="v")
            for jt in range(JT):
                half, col = divmod(jt, JT // 2)
                # dma_start_transpose corrupts non-zero-offset dst slices;
                # bounce through a contiguous tile (see tile_matmul.py note)
                vtmp = attp.tile([P, H], BF, tag="vtmp")
                nc.sync.dma_start_transpose(
                    vtmp, vgath[ds(half * H, H), ds(col * P, P)])
                nc.vector.tensor_copy(v_sb[:, jt, 0:H], vtmp)
                nc.vector.memset(v_sb[:, jt, H:], 1.0)

            # ---- q projection (padded to 128 partitions) ----
            q_sb = res.tile([P, NQ], BF, tag="q")
            nc.vector.memset(q_sb[H:, :], 0.0)
            mask_idx = {sj: i for i, sj in enumerate(masked_slots)}

            # ---- per slot: q proj, then attention ----
            for s in range(NSLOTS):
                pq_full = pp.tile([H, 512], FP, tag="pp", name="pq")
                pq = pq_full[:, :CHUNK]
                for d in range(DCH):
                    nc.tensor.matmul(pq, lhsT=wq_sb[:, d, :],
                                     rhs=xq_sb[:, d, ds(s * CHUNK, CHUNK)],
                                     start=(d == 0), stop=(d == DCH - 1))
                nc.scalar.activation(q_sb[:H, ds(s * CHUNK, CHUNK)], pq,
                                     mybir.ActivationFunctionType.Identity,
                                     bias=bq_sb)

                po = pout.tile([H + 1, CHUNK], FP, tag="po")
                ext = extents[s]
                for jt in range(ext):
                    ps = psc.tile([P, CHUNK], FP, tag="sc")
                    nc.tensor.matmul(ps, lhsT=k_sb[:, ds(jt * P, P)],
                                     rhs=q_sb[:, ds(s * CHUNK, CHUNK)],
                                     start=True, stop=True)
                    att = attp.tile([P, CHUNK], BF, tag="att")
                    nc.scalar.activation(att, ps,
                                         mybir.ActivationFunctionType.Exp,
                                         scale=0.125)
                    mi = mask_idx.get((s, jt))
                    if mi is not None:
                        nc.vector.copy_predicated(att, pred_sb[:, mi, :],
                                                  zeros_sb)
                    nc.tensor.matmul(po, lhsT=v_sb[:, jt, :], rhs=att,
                                     start=(jt == 0), stop=(jt == ext - 1))
                # epilogue: transpose + normalize into the staging tile
                oT_sb = outp.tile([P, CHUNK], FP, tag="oT")
                nc.vector.tensor_copy(oT_sb[:H + 1, :], po)
                for t in range(CHUNK // P):
                    pt = ptr.tile([P, P], FP, tag="tr")
                    nc.tensor.transpose(pt, oT_sb[:, ds(t * P, P)], ident)
                    recip = outp.tile([P, 1], FP, tag="recip")
                    nc.vector.reciprocal(recip, pt[:, H:H + 1])
                    o_sb = outp.tile([P, H], FP, tag="o", bufs=2)
                    nc.vector.tensor_scalar_mul(o_sb, pt[:, :H], recip)
                    nc.sync.dma_start(
                        out_d[ds(s * CHUNK + t * P, P), :], o_sb)

            if debug:
                nc.sync.dma_start(dbg_k, k_sb)
                nc.sync.dma_start(dbg_v, v_sb)
                nc.sync.dma_start(dbg_q, q_sb)

    nc.compile()
    return nc


def _mask_fits_causal_variant(mask):
    """Check the causal variant computes every allowed position and that
    skipped/unpredicated regions match the mask."""
    # per-chunk computed bound (keys) and fully-allowed bound
    for h, chunks in CAUSAL_CHUNKS.items():
        for s, g in enumerate(chunks):
            rows = slice(g * CHUNK, (g + 1) * CHUNK)
            bound = CAUSAL_EXTENTS[s] * P
            lo = 4 * s * P  # below this, no predicate is applied
            if bound < S and mask[:, rows, bound:].any():
                return False
            if lo > 0 and not mask[:, rows, :lo].all():
                return False
    return True


def kernel(query, key, value, mask, Wq, bq, Wk, bk, Wv, bv):
    global LAST_RESULTS
    query = np.asarray(query, dtype=np.float32)
    key = np.asarray(key, dtype=np.float32)
    value = np.asarray(value, dtype=np.float32)
    mask = np.asarray(mask).astype(bool)
    Wq = np.asarray(Wq, dtype=np.float32)
    Wk = np.asarray(Wk, dtype=np.float32)
    Wv = np.asarray(Wv, dtype=np.float32)
    bq = np.asarray(bq, dtype=np.float32)
    bk = np.asarray(bk, dtype=np.float32)
    bv = np.asarray(bv, dtype=np.float32)

    if _mask_fits_causal_variant(mask):
        key_v = "causal"
        extents, chunks_of, masked = CAUSAL_EXTENTS, CAUSAL_CHUNKS, CAUSAL_MASKED
    else:
        key_v = "full"
        extents, chunks_of, masked = FULL_EXTENTS, FULL_CHUNKS, FULL_MASKED

    if key_v not in _PROGRAM_CACHE:
        _PROGRAM_CACHE[key_v] = _build_program(extents, masked)
    nc = _PROGRAM_CACHE[key_v]

    wq_bf = Wq.astype(BF_NP)
    wk_bf = Wk.astype(BF_NP)
    wv_bf = Wv.astype(BF_NP)
    bq_in = bq.reshape(H, 1)
    bk_in = bk.reshape(H, 1)
    bv_in = bv.reshape(H, 1)

    in_maps = []
    for c in range(NCORES):
        b, h = divmod(c, 2)
        chunks = chunks_of[h]
        q_rows = np.concatenate(
            [query[b, g * CHUNK:(g + 1) * CHUNK, :] for g in chunks], axis=0)
        qT = np.ascontiguousarray(q_rows.T).astype(BF_NP)
        kT = np.ascontiguousarray(key[b, h * SHALF:(h + 1) * SHALF, :].T
                                  ).astype(BF_NP)
        vT = np.ascontiguousarray(value[b, h * SHALF:(h + 1) * SHALF, :].T
                                  ).astype(BF_NP)
        pred = np.zeros((len(masked), P, CHUNK), dtype=np.uint8)
        for i, (s, jt) in enumerate(masked):
            g = chunks[s]
            blk = mask[b, g * CHUNK:(g + 1) * CHUNK,
                       jt * P:(jt + 1) * P]          # [i=256, j=128]
            pred[i] = (~blk.T).astype(np.uint8)       # [j=128, i=256]
        in_maps.append({
            "qT": qT, "kT": kT, "vT": vT,
            "wq": wq_bf, "wk": wk_bf, "wv": wv_bf,
            "bq": bq_in, "bk": bk_in, "bv": bv_in,
            "pred": pred,
        })

    results = run_bass_kernel_spmd(
        nc, in_maps, core_ids=list(range(NCORES)),
        trace=bool(os.environ.get("BASS_TRACE")),
    )
    LAST_RESULTS = results

    out = np.empty((B, S, H), dtype=np.float32)
    for c in range(NCORES):
        b, h = divmod(c, 2)
        chunks = chunks_of[h]
        o = results.results[c]["out"]
        for s, g in enumerate(chunks):
            out[b, g * CHUNK:(g + 1) * CHUNK, :] = \
                o[s * CHUNK:(s + 1) * CHUNK]
    return out
